# revision 38
# baseline (speedup 1.0000x reference)
"""Trainium2 Bass kernel for the additive-attention (tanh) unit.

Computes, for full inputs:
    tq   = queries @ Ww.T + Wb                  (S,B,A)
    tk   = keys    @ Uw.T + Ub                  (B,K,A)
    alpha[s,b,k] = sum_a v[a]*tanh(tq[s,b,a]+tk[b,k,a]) + vb
    alpha_masked = where(null_mask, -inf, alpha)
    att  = softmax_k(alpha_masked) @ keys       (S,B,KD)
returns (att, alpha_masked).

Strategy: shard batch B=16 across 8 cores (2 per core).  The O(S*B*K*A)
tanh is evaluated through a separable expansion
    tanh(x+y) ~= a0*(x+y) + sum_m c_m sin(w_m (x+y))
    sin(w(x+y)) = sin(wx)cos(wy) + cos(wx)sin(wy)
so the huge elementwise pass becomes 2M ScalarE sine passes over the
(concatenated) tq/tk tensors plus bf16 TensorE matmuls contracting over
A.  ACT Sin is only accurate to |arg| ~3.6 rad, so sine arguments are
range-reduced on VectorE with ADD_RANGE_WRAP chains (period 2*pi/w_m in
t-units); cosines reuse the sin chain via a +bound/2-shifted wrap, or a
+pi/2 bias directly when the coefficient is small enough to tolerate the
spline's extended-range error.  The linear a0 term contracts host-
precomputed a0*(v@Ww) / a0*(v@Uw) vectors against the raw inputs; its
k-independent part rides the per-row alpha bias.  The sine fit is a
minimax (Lawson) fit of tanh on [-Z0, Z0] with Z0 chosen per call from
the exact max|tq|+max|tk| (wider fallback tables are baked in).  The
masked softmax (exp via ScalarE with fused row-max bias, zeroing by a
0/1 mask with a fused accumulate-sum) and the final att_weight @ keys
matmul (PE transposes + bf16 matmuls) run on device; the -inf entries
of the alpha output are stamped on the host from the input mask during
output assembly.
"""

import math

import numpy as np
import ml_dtypes

S, B, K = 64, 16, 512
QD, KD, A = 512, 512, 256
NCORES = 8
BL = B // NCORES  # local batches per core
HALF_PI = math.pi / 2.0

# Minimax sine+linear fits of tanh(z) on [-Z0, Z0]:  (omegas, coefs, a0)
# tanh(z) ~= a0*z + sum_m coefs[m]*sin(omegas[m]*z),  max abs err as noted.
TANH_SIN_TABLES = [
    # (Z0, omegas, coefs, a0)
    (5.6,
     [0.8565121028227215, 1.7527802239290013, 2.703340898746608,
      3.697215236619702],
     [0.48681900456852495, 0.11874081052415719, 0.028199252821259842,
      0.005817125535546059],
     0.26988587080463133),  # M=4 maxerr 1.84e-3
    (7.0,
     [0.7162402410602733, 1.4531797472781942, 2.220103962022262,
      3.0165100097340867, 3.8271148337656116],
     [0.5249758484918143, 0.15512641288141396, 0.04846434735631873,
      0.013815641577942473, 0.003764714878953481],
     0.22657329081408695),  # M=5 maxerr 1.66e-3
    (9.0,
     [0.5787400036036481, 1.166853458146944, 1.7695626949997472,
      2.388033887818974, 3.021261461002983, 3.6661316403846085,
      4.307593305397026],
     [0.5597857380561834, 0.19527061675232465, 0.0762222687663923,
      0.0296326807916147, 0.011107975611302298, 0.0039022212116882663,
      0.0014877552796711906],
     0.18359419872632915),  # M=7 maxerr 8.6e-4
    (12.0,
     [0.4529040795961431, 0.9095058780434189, 1.3722870887643785,
      1.8422590864823205, 2.3194193469181115, 2.8031940201778203,
      3.2924279661277325, 3.7841959840549975, 4.2651261028563],
     [0.5874323504319142, 0.2335119271470687, 0.10951468933716764,
      0.05272837478693417, 0.025315364944069395, 0.011960636409267545,
      0.005485635950776176, 0.0024466372890161155, 0.0013150351469835761],
     0.14389597954049463),  # M=9 maxerr 9.5e-4
]

_program_cache: dict = {}


def _build_program(pbias_val: float, table_idx: int,
                   tmax_q: float = 3.3, tmax_k: float = 3.5):
    import concourse.bass as bass
    import concourse.tile as tile
    from concourse import bacc, mybir
    from concourse import masks

    _, omegas, coefs, _a0 = TANH_SIN_TABLES[table_idx]
    M = len(omegas)

    f32 = mybir.dt.float32
    bf16 = mybir.dt.bfloat16
    AF = mybir.ActivationFunctionType
    OP = mybir.AluOpType
    AX = mybir.AxisListType

    nc = bacc.Bacc(
        "TRN2",
        target_bir_lowering=False,
        debug=False,
        enable_asserts=False,
        num_devices=NCORES,
    )

    # ---- DRAM I/O -------------------------------------------------------
    qT_d = nc.dram_tensor("qT", (BL, QD, S), bf16, kind="ExternalInput").ap()
    kT_d = nc.dram_tensor("kT", (BL, KD, K), bf16, kind="ExternalInput").ap()
    keys_d = nc.dram_tensor("keysb", (BL, K, KD), bf16, kind="ExternalInput").ap()
    WwT_d = nc.dram_tensor("WwT", (QD, A), bf16, kind="ExternalInput").ap()
    UwT_d = nc.dram_tensor("UwT", (KD, A), bf16, kind="ExternalInput").ap()
    Wb_d = nc.dram_tensor("Wbc", (A, 1), f32, kind="ExternalInput").ap()
    Ub_d = nc.dram_tensor("Ubc", (A, 1), f32, kind="ExternalInput").ap()
    vrep_d = nc.dram_tensor("vrep", (128, 2, A // 128, BL, S), bf16,
                            kind="ExternalInput").ap()
    wlin_d = nc.dram_tensor("wlin", (128, QD // 128), bf16,
                            kind="ExternalInput").ap()
    ulin_d = nc.dram_tensor("ulin", (128, KD // 128, S), bf16,
                            kind="ExternalInput").ap()
    m01_d = nc.dram_tensor("mask01", (S, BL, K), bf16, kind="ExternalInput").ap()
    att_d = nc.dram_tensor("att_out", (S, BL, KD), f32, kind="ExternalOutput").ap()
    alp_d = nc.dram_tensor("alpha_out", (S, BL, K), f32, kind="ExternalOutput").ap()

    AH = A // 128  # 2 partition tiles over the A axis

    with tile.TileContext(nc) as tc:
        with (
            tc.tile_pool(name="consts", bufs=1) as cpool,
            tc.tile_pool(name="sb", bufs=1) as spool,
            tc.tile_pool(name="ftk", bufs=2) as fpool,
            tc.tile_pool(name="gtq", bufs=2) as gpool,
            tc.tile_pool(name="psA", bufs=1, space="PSUM") as psA,
            tc.tile_pool(name="psK", bufs=2, space="PSUM") as psK,
            tc.tile_pool(name="ps1", bufs=1, space="PSUM") as ps1,
        ):
            # ---- constants / small tiles --------------------------------
            idt = cpool.tile([128, 128], bf16, tag="idt")
            masks.make_identity(nc, idt[:])
            phc = cpool.tile([128, 1], f32, tag="phc")
            nc.vector.memset(phc[:], HALF_PI)
            warm = cpool.tile([128, 1], f32, tag="warm")
            nc.scalar.activation(warm[:], phc[:], AF.Sin, scale=1.0)
            wps = ps1.tile([128, 128], f32, tag="attps", bufs=2, name="wps")
            for i in range(16):
                nc.tensor.matmul(wps[:], idt[:], idt[:],
                                 start=(i == 0), stop=(i == 15))


            Wbs = cpool.tile([128, AH], f32, tag="Wbs")
            nc.gpsimd.dma_start(Wbs[:], Wb_d.rearrange("(ah p) one -> p (ah one)", p=128))
            Ubs = cpool.tile([128, AH], f32, tag="Ubs")
            nc.gpsimd.dma_start(Ubs[:], Ub_d.rearrange("(ah p) one -> p (ah one)", p=128))
            vreps = cpool.tile([128, 2, AH, BL, S], bf16, tag="vreps")
            nc.gpsimd.dma_start(vreps[:], vrep_d[:])
            wlin = cpool.tile([128, QD // 128], bf16, tag="wlin")
            nc.gpsimd.dma_start(wlin[:], wlin_d[:])
            ulin = cpool.tile([128, KD // 128, S], bf16, tag="ulin")
            nc.gpsimd.dma_start(ulin[:], ulin_d[:])

            # ---- bulk inputs (b0 tk chain first: it gates the pipeline)
            UwTs = cpool.tile([128, KD // 128, A], bf16, tag="UwTs")
            nc.scalar.dma_start(UwTs[:], UwT_d.rearrange("(dh p) a -> p dh a", p=128))
            kTs = spool.tile([128, KD // 128, BL, K], bf16, tag="kTs")
            for dh in range(KD // 128):
                eng = nc.sync if (dh % 2 == 0) else nc.scalar
                eng.dma_start(kTs[:, dh, 0, :], kT_d[0, dh * 128:(dh + 1) * 128, :])
            WwTs = cpool.tile([128, QD // 128, A], bf16, tag="WwTs")
            nc.scalar.dma_start(WwTs[:], WwT_d.rearrange("(qh p) a -> p qh a", p=128))
            qTs = spool.tile([128, QD // 128, BL, S], bf16, tag="qTs")
            for b in range(BL):
                nc.scalar.dma_start(
                    qTs[:, :, b, :],
                    qT_d[b].rearrange("(qh p) s -> p qh s", p=128))
            for dh in range(KD // 128):
                eng = nc.sync if (dh % 2 == 0) else nc.scalar
                eng.dma_start(kTs[:, dh, 1, :], kT_d[1, dh * 128:(dh + 1) * 128, :])
            keysb = spool.tile([128, K // 128, BL, KD], bf16, tag="keysb")
            for b in range(BL):
                for kh in range(K // 128):
                    nc.gpsimd.dma_start(
                        keysb[:, kh, b, :],
                        keys_d[b, kh * 128:(kh + 1) * 128, :])
            m01 = spool.tile([S, BL, K], bf16, tag="m01")
            nc.gpsimd.dma_start(m01[:], m01_d[:])

            # ---- combined t tensor: t[:, ah, b, :K] = tk, [K:] = tq ------
            # (single tensor so each range-wrap / sine pass covers both
            # sides in one instruction)
            KS = K + S
            tcomb = spool.tile([128, AH, BL, KS], f32, tag="tcomb")
            tk = tcomb[:, :, :, :K]
            tq = tcomb[:, :, :, K:]
            for b in range(BL):
                for ah in range(AH):
                    ps = ps1.tile([128, S], f32, tag="tqps", bufs=2)
                    for qh in range(QD // 128):
                        nc.tensor.matmul(
                            ps[:],
                            WwTs[:, qh, ah * 128:(ah + 1) * 128],
                            qTs[:, qh, b, :],
                            start=(qh == 0), stop=(qh == QD // 128 - 1),
                        )
                    nc.vector.tensor_scalar_add(tq[:, ah, b, :], ps[:],
                                                Wbs[:, ah:ah + 1])

            # ---- P[s] = a0 * sum_a v_a tq[s,a]  (+vb), per b ------------
            Psb = []
            for b in range(BL):
                pps = ps1.tile([S, 1], f32, tag="tqps", bufs=2, name="pps")
                for qh in range(QD // 128):
                    nc.tensor.matmul(
                        pps[:],
                        qTs[:, qh, b, :],
                        wlin[:, qh:qh + 1],
                        start=(qh == 0), stop=(qh == QD // 128 - 1),
                    )
                pb = spool.tile([S, 1], f32, tag=f"Psb{b}", name=f"Psb{b}")
                nc.vector.tensor_scalar_add(pb[:], pps[:], float(pbias_val))
                Psb.append(pb)

            # ---- tk = Uw @ keys^T (+Ub), laid out (a, k) per b ----------
            for b in range(BL):
                for ah in range(AH):
                    psk = psK.tile([128, K], f32, tag="tkps")
                    for dh in range(KD // 128):
                        nc.tensor.matmul(
                            psk[:],
                            UwTs[:, dh, ah * 128:(ah + 1) * 128],
                            kTs[:, dh, b, :],
                            start=(dh == 0), stop=(dh == KD // 128 - 1),
                        )
                    nc.scalar.add(tk[:, ah, b, :], psk[:],
                                  Ubs[:, ah:ah + 1])

            # ---- alpha accumulation in PSUM, (s, k) per b ---------------
            alpha_ps = [psA.tile([S, K], f32, name=f"al{b}", tag=f"al{b}")
                        for b in range(BL)]

            # linear term:  alpha += a0 * sum_a v_a tk[k,a]  (via raw keys)
            for b in range(BL):
                for dh in range(KD // 128):
                    nc.tensor.matmul(
                        alpha_ps[b][:],
                        ulin[:, dh, :],
                        kTs[:, dh, b, :],
                        start=(dh == 0), stop=False,
                    )

            # sine terms.  ACT Sin is only accurate to |arg| ~3.6-3.7 rad, so
            # arguments are range-reduced with ADD_RANGE_WRAP chains applied
            # to the raw t-tensors (bound pi/w, period 2pi/w in t-units).
            # The cos chain wraps (t + bound/2), so sin(w*u_c) = cos(w*t).
            # For tiny coefficients the cos is taken directly from u_s with a
            # +pi/2 bias (|arg| <= pi + pi/2 where the sin spline error,
            # ~0.075 absolute, is still small enough after weighting by c_m).
            tmax = max(tmax_q, tmax_k)

            def wrap_chain(src_view, w_m, tag, with_shift, shape):
                # src_view/dst used via flattened free dims (custom-DVE ops
                # accept <=2 free dims)
                bnd = math.pi / w_m
                if with_shift:
                    nwrap = 1
                else:
                    if w_m * tmax <= 3.6:
                        return src_view, 0
                    nwrap = max(1, int(math.ceil((tmax / bnd - 1.0) / 2.0
                                                 - 1e-9)))
                cur = src_view
                for i in range(nwrap):
                    dst = fpool.tile(shape, f32,
                                     tag=f"{tag}{i}", name=f"{tag}{i}",
                                     bufs=(1 if len(shape) == 3 else None))
                    sh = (bnd / 2.0) if (with_shift and i == 0) else 0.0
                    if len(shape) == 4:
                        d_v = dst[:].rearrange("p a b k -> p (a b k)")
                        c_v = cur[:].rearrange("p a b k -> p (a b k)")
                    else:
                        d_v, c_v = dst[:], cur[:]  # (p, a, k): 2 free dims
                    nc.vector.add_range_wrap(d_v, c_v, sh, bnd, 2.0 * bnd)
                    cur = dst
                return cur, nwrap

            def emit_m(m, bs, stop_b=None):
                """Emit one harmonic's wraps/sines/scale/matmuls for batches
                `bs` (either all of them in one shot, or a single b)."""
                w_m = float(omegas[m])
                c_m = float(coefs[m])
                cos_direct = abs(c_m) <= 0.03
                split = len(bs) < BL
                sfx = f"b{bs[0]}" if split else ""
                if split:
                    src = tcomb[:, :, bs[0], :]
                    shape = [128, AH, KS]
                else:
                    src = tcomb
                    shape = [128, AH, BL, KS]
                u_s, _ = wrap_chain(src, w_m, "uks" + sfx, False, shape)
                fshape = [128, 2] + shape[1:]
                fsc = fpool.tile(fshape, bf16, tag="fsc" + sfx,
                                 name="fsc" + sfx,
                                 bufs=(1 if split else None))
                nc.scalar.activation(fsc[:, 0], u_s[:], AF.Sin, scale=w_m)
                if cos_direct:
                    nc.scalar.activation(fsc[:, 1], u_s[:], AF.Sin,
                                         scale=w_m, bias=phc[:])
                else:
                    u_c, _ = wrap_chain(u_s, w_m, "ukc" + sfx, True, shape)
                    nc.scalar.activation(fsc[:, 1], u_c[:], AF.Sin,
                                         scale=w_m)
                gshape = fshape[:-1] + [S]
                gsc = gpool.tile(gshape, bf16, tag="gsc" + sfx,
                                 name="gsc" + sfx,
                                 bufs=(1 if split else None))
                if split:
                    fq = fsc[:, :, :, K:]
                    vr = vreps[:, :, :, bs[0], :]
                else:
                    fq = fsc[:, :, :, :, K:]
                    vr = vreps[:]
                nc.vector.scalar_tensor_tensor(gsc[:], fq, c_m, vr,
                                               op0=OP.mult, op1=OP.mult)
                for b in bs:
                    for ah in range(AH):
                        if split:
                            g0, g1 = gsc[:, 0, ah, :], gsc[:, 1, ah, :]
                            f0, f1 = fsc[:, 0, ah, :K], fsc[:, 1, ah, :K]
                        else:
                            g0, g1 = gsc[:, 0, ah, b, :], gsc[:, 1, ah, b, :]
                            f0 = fsc[:, 0, ah, b, :K]
                            f1 = fsc[:, 1, ah, b, :K]
                        stop = (stop_b == b and ah == AH - 1)
                        nc.tensor.matmul(alpha_ps[b][:], g0, f1,
                                         start=False, stop=False)
                        nc.tensor.matmul(alpha_ps[b][:], g1, f0,
                                         start=False, stop=stop)

            # PE "heater" blocks: back-to-back tiny matmuls emitted between
            # harmonic groups.  They run whenever the next real matmul's
            # operands aren't ready yet, keeping the PE busy through the
            # ACT/DVE-wait gaps so the HAM clock gate stays at full rate
            # (throttled alpha matmuls cost 427-788ns vs 213ns warm).
            def pe_fill(n, j):
                fps = ps1.tile([128, 128], f32, tag="attps", bufs=2,
                               name=f"fill{j}")
                for i in range(n):
                    nc.tensor.matmul(fps[:], idt[:], idt[:],
                                     start=(i == 0), stop=(i == n - 1))

            # m=0 split per b (primes the pipeline as soon as each batch's
            # t-tensor is ready); middle m's batched; last m split per b so
            # b0's softmax overlaps b1's remaining work.
            emit_m(0, [0])
            emit_m(0, [1])
            pe_fill(34, 0)
            for m in range(1, M - 1):
                emit_m(m, list(range(BL)))
                pe_fill(34, m)
            emit_m(M - 1, [0], stop_b=0)
            emit_m(M - 1, [1], stop_b=1)
            # dummy exp on a const tile: triggers the exp table load while
            # the last alpha matmuls are still in flight
            nc.scalar.activation(warm[:], phc[:], AF.Exp)

            # ---- softmax + outputs per b --------------------------------
            for b in range(BL):
                nmax = spool.tile([S, 1], f32, tag=f"nmax{b}")
                nc.vector.tensor_reduce(nmax[:], alpha_ps[b][:], axis=AX.X,
                                        op=OP.max, negate=True)
                esb = spool.tile([S, K], bf16, tag=f"esb{b}")
                nc.scalar.activation(esb[:], alpha_ps[b][:], AF.Exp,
                                     bias=nmax[:], scale=1.0)
                expm = spool.tile([S, K], bf16, tag=f"expm{b}")
                sume = spool.tile([S, 1], f32, tag=f"sume{b}")
                nc.vector.scalar_tensor_tensor(expm[:], esb[:], 1.0,
                                               m01[:, b, :],
                                               op0=OP.mult, op1=OP.mult,
                                               accum_out=sume[:])
                rcp = spool.tile([S, 1], f32, tag=f"rcp{b}")
                nc.vector.reciprocal(rcp[:], sume[:])

                # alpha output = alpha + P[s] (+vb); -inf mask applied on
                # the host during output assembly
                alpsb = spool.tile([S, K], f32, tag=f"alp{b}")
                nc.scalar.activation(alpsb[:], alpha_ps[b][:], AF.Identity,
                                     bias=Psb[b][:])
                nc.scalar.dma_start(alp_d[:, b, :], alpsb[:])

                # transpose masked weights to (k, s)
                wT = spool.tile([128, K // 128, S], bf16, tag=f"wT{b}")
                for kh in range(K // 128):
                    tps = ps1.tile([128, S], bf16, tag="tqps", bufs=2, name="tps")
                    nc.tensor.transpose(tps[:],
                                        expm[:, kh * 128:(kh + 1) * 128],
                                        idt[:S, :S])
                    nc.vector.tensor_copy(wT[:, kh, :], tps[:])

                att_ps = ps1.tile([S, KD], f32, tag="attps", bufs=2)
                for kh in range(K // 128):
                    nc.tensor.matmul(
                        att_ps[:],
                        wT[:, kh, :],
                        keysb[:, kh, b, :],
                        start=(kh == 0), stop=(kh == K // 128 - 1),
                    )
                attsb = spool.tile([S, KD], f32, tag=f"att{b}")
                nc.scalar.activation(attsb[:], att_ps[:], AF.Copy,
                                     scale=rcp[:])
                nc.sync.dma_start(att_d[:, b, :], attsb[:])

    nc.compile()
    return nc


def _pick_table(zmax: float) -> int:
    for i, (z0, _, _, _) in enumerate(TANH_SIN_TABLES):
        if zmax <= z0:
            return i
    return len(TANH_SIN_TABLES) - 1


def kernel(queries, keys, null_mask, Ww, Wb, Uw, Ub, vw, vb):
    from concourse import bass_utils

    queries = np.asarray(queries, dtype=np.float32)
    keys = np.asarray(keys, dtype=np.float32)
    null_mask = np.asarray(null_mask)
    Ww = np.asarray(Ww, dtype=np.float32)
    Wb = np.asarray(Wb, dtype=np.float32)
    Uw = np.asarray(Uw, dtype=np.float32)
    Ub = np.asarray(Ub, dtype=np.float32)
    vw = np.asarray(vw, dtype=np.float32)
    vb = np.asarray(vb, dtype=np.float32)
    v = vw[0]  # (A,)
    vb_val = float(vb[0])

    # Host-side range check to pick the sine table (exact max|tq+tk| via
    # per-(b,a) extrema of tq over s and tk over k).
    tq_h = (queries.reshape(S * B, QD) @ Ww.T).reshape(S, B, A) + Wb
    tk_h = (keys.reshape(B * K, KD) @ Uw.T).reshape(B, K, A) + Ub
    zmax = max(
        float((tq_h.max(0) + tk_h.max(1)).max()),
        float(-(tq_h.min(0) + tk_h.min(1)).min()),
    )
    tidx = _pick_table(zmax)
    z0, omegas, coefs, a0 = TANH_SIN_TABLES[tidx]

    # alpha linear-term constants that are uniform over k fold into the
    # per-row bias: vb + a0*(v.Wb) + a0*(v.Ub)
    pbias = vb_val + float(a0 * (v @ Wb)) + float(a0 * (v @ Ub))

    tmax_q = float(np.abs(tq_h).max()) * 1.02
    tmax_k = float(np.abs(tk_h).max()) * 1.02
    key_ = (round(pbias, 12), tidx, round(tmax_q, 2), round(tmax_k, 2))
    if key_ not in _program_cache:
        _program_cache[key_] = _build_program(pbias, tidx, tmax_q, tmax_k)
    nc = _program_cache[key_]

    bf = ml_dtypes.bfloat16
    WwT = np.ascontiguousarray(Ww.T).astype(bf)          # (QD, A)
    UwT = np.ascontiguousarray(Uw.T).astype(bf)          # (KD, A)
    Wb_c = np.ascontiguousarray(Wb.reshape(A, 1))
    Ub_c = np.ascontiguousarray(Ub.reshape(A, 1))
    v_pa = np.ascontiguousarray(v.reshape(A // 128, 128).T)      # (128, AH)
    vrep = np.ascontiguousarray(
        np.broadcast_to(v_pa[None, :, :, None, None],
                        (2, 128, A // 128, BL, S)).transpose(1, 0, 2, 3, 4)
    ).astype(bf)
    wlin_v = (a0 * (v @ Ww)).reshape(QD // 128, 128).T   # (128, qh)
    wlin = np.ascontiguousarray(wlin_v).astype(bf)
    ulin_v = (a0 * (v @ Uw)).reshape(KD // 128, 128).T   # (128, dh)
    ulin = np.ascontiguousarray(
        np.broadcast_to(ulin_v[:, :, None], (128, KD // 128, S))
    ).astype(bf)

    in_maps = []
    for c in range(NCORES):
        bs = slice(c * BL, (c + 1) * BL)
        q_c = queries[:, bs, :]                       # (S, BL, QD)
        k_c = keys[bs]                                # (BL, K, KD)
        nm_c = null_mask[bs]                          # (BL, K)
        m01 = np.ascontiguousarray(
            np.broadcast_to((~nm_c).astype(bf)[None, :, :], (S, BL, K))
        )
        in_maps.append({
            "qT": np.ascontiguousarray(q_c.transpose(1, 2, 0)).astype(bf),
            "kT": np.ascontiguousarray(k_c.transpose(0, 2, 1)).astype(bf),
            "keysb": np.ascontiguousarray(k_c).astype(bf),
            "WwT": WwT,
            "UwT": UwT,
            "Wbc": Wb_c,
            "Ubc": Ub_c,
            "vrep": vrep,
            "wlin": wlin,
            "ulin": ulin,
            "mask01": m01,
        })

    res = bass_utils.run_bass_kernel_spmd(
        nc, in_maps, core_ids=list(range(NCORES)))
    global last_result
    last_result = res

    att = np.empty((S, B, KD), np.float32)
    alpha = np.empty((S, B, K), np.float32)
    for c in range(NCORES):
        bs = slice(c * BL, (c + 1) * BL)
        att[:, bs, :] = res.results[c]["att_out"]
        alpha[:, bs, :] = res.results[c]["alpha_out"]
    alpha[np.broadcast_to(null_mask[None, :, :], alpha.shape)] = -np.inf
    return att, alpha


# revision 39
# speedup vs baseline: 1.0573x; 1.0573x over previous
"""Trainium2 Bass kernel for the additive-attention (tanh) unit.

Computes, for full inputs:
    tq   = queries @ Ww.T + Wb                  (S,B,A)
    tk   = keys    @ Uw.T + Ub                  (B,K,A)
    alpha[s,b,k] = sum_a v[a]*tanh(tq[s,b,a]+tk[b,k,a]) + vb
    alpha_masked = where(null_mask, -inf, alpha)
    att  = softmax_k(alpha_masked) @ keys       (S,B,KD)
returns (att, alpha_masked).

Strategy: shard batch B=16 across 8 cores (2 per core).  The O(S*B*K*A)
tanh is evaluated through a separable expansion
    tanh(x+y) ~= a0*(x+y) + sum_m c_m sin(w_m (x+y))
    sin(w(x+y)) = sin(wx)cos(wy) + cos(wx)sin(wy)
so the huge elementwise pass becomes 2M ScalarE sine passes over the
(concatenated) tq/tk tensors plus bf16 TensorE matmuls contracting over
A.  ACT Sin is only accurate to |arg| ~3.6 rad, so sine arguments are
range-reduced on VectorE with ADD_RANGE_WRAP chains (period 2*pi/w_m in
t-units); cosines reuse the sin chain via a +bound/2-shifted wrap, or a
+pi/2 bias directly when the coefficient is small enough to tolerate the
spline's extended-range error.  The linear a0 term contracts host-
precomputed a0*(v@Ww) / a0*(v@Uw) vectors against the raw inputs; its
k-independent part rides the per-row alpha bias.  The sine fit is a
minimax (Lawson) fit of tanh on [-Z0, Z0] with Z0 chosen per call from
the exact max|tq|+max|tk| (wider fallback tables are baked in).  The
masked softmax (exp via ScalarE with fused row-max bias, zeroing by a
0/1 mask with a fused accumulate-sum) and the final att_weight @ keys
matmul (PE transposes + bf16 matmuls) run on device; the -inf entries
of the alpha output are stamped on the host from the input mask during
output assembly.
"""

import math

import numpy as np
import ml_dtypes

S, B, K = 64, 16, 512
QD, KD, A = 512, 512, 256
NCORES = 8
BL = B // NCORES  # local batches per core
HALF_PI = math.pi / 2.0

# Minimax sine+linear fits of tanh(z) on [-Z0, Z0]:  (omegas, coefs, a0)
# tanh(z) ~= a0*z + sum_m coefs[m]*sin(omegas[m]*z),  max abs err as noted.
TANH_SIN_TABLES = [
    # (Z0, omegas, coefs, a0)
    (5.6,
     [0.8565121028227215, 1.7527802239290013, 2.703340898746608,
      3.697215236619702],
     [0.48681900456852495, 0.11874081052415719, 0.028199252821259842,
      0.005817125535546059],
     0.26988587080463133),  # M=4 maxerr 1.84e-3
    (7.0,
     [0.7162402410602733, 1.4531797472781942, 2.220103962022262,
      3.0165100097340867, 3.8271148337656116],
     [0.5249758484918143, 0.15512641288141396, 0.04846434735631873,
      0.013815641577942473, 0.003764714878953481],
     0.22657329081408695),  # M=5 maxerr 1.66e-3
    (9.0,
     [0.5787400036036481, 1.166853458146944, 1.7695626949997472,
      2.388033887818974, 3.021261461002983, 3.6661316403846085,
      4.307593305397026],
     [0.5597857380561834, 0.19527061675232465, 0.0762222687663923,
      0.0296326807916147, 0.011107975611302298, 0.0039022212116882663,
      0.0014877552796711906],
     0.18359419872632915),  # M=7 maxerr 8.6e-4
    (12.0,
     [0.4529040795961431, 0.9095058780434189, 1.3722870887643785,
      1.8422590864823205, 2.3194193469181115, 2.8031940201778203,
      3.2924279661277325, 3.7841959840549975, 4.2651261028563],
     [0.5874323504319142, 0.2335119271470687, 0.10951468933716764,
      0.05272837478693417, 0.025315364944069395, 0.011960636409267545,
      0.005485635950776176, 0.0024466372890161155, 0.0013150351469835761],
     0.14389597954049463),  # M=9 maxerr 9.5e-4
]

_program_cache: dict = {}


def _build_program(pbias_val: float, table_idx: int,
                   tmax_q: float = 3.3, tmax_k: float = 3.5):
    import concourse.bass as bass
    import concourse.tile as tile
    from concourse import bacc, mybir
    from concourse import masks

    _, omegas, coefs, _a0 = TANH_SIN_TABLES[table_idx]
    M = len(omegas)

    f32 = mybir.dt.float32
    bf16 = mybir.dt.bfloat16
    AF = mybir.ActivationFunctionType
    OP = mybir.AluOpType
    AX = mybir.AxisListType

    nc = bacc.Bacc(
        "TRN2",
        target_bir_lowering=False,
        debug=False,
        enable_asserts=False,
        num_devices=NCORES,
    )

    # ---- DRAM I/O -------------------------------------------------------
    qT_d = nc.dram_tensor("qT", (BL, QD, S), bf16, kind="ExternalInput").ap()
    kT_d = nc.dram_tensor("kT", (BL, KD, K), bf16, kind="ExternalInput").ap()
    keys_d = nc.dram_tensor("keysb", (BL, K, KD), bf16, kind="ExternalInput").ap()
    WwT_d = nc.dram_tensor("WwT", (QD, A), bf16, kind="ExternalInput").ap()
    UwT_d = nc.dram_tensor("UwT", (KD, A), bf16, kind="ExternalInput").ap()
    Wb_d = nc.dram_tensor("Wbc", (A, 1), f32, kind="ExternalInput").ap()
    Ub_d = nc.dram_tensor("Ubc", (A, 1), f32, kind="ExternalInput").ap()
    vrep_d = nc.dram_tensor("vrep", (128, 2, A // 128, BL, S), bf16,
                            kind="ExternalInput").ap()
    wlin_d = nc.dram_tensor("wlin", (128, QD // 128), bf16,
                            kind="ExternalInput").ap()
    ulin_d = nc.dram_tensor("ulin", (128, KD // 128, S), bf16,
                            kind="ExternalInput").ap()
    m01_d = nc.dram_tensor("mask01", (S, BL, K), bf16, kind="ExternalInput").ap()
    att_d = nc.dram_tensor("att_out", (S, BL, KD), f32, kind="ExternalOutput").ap()
    alp_d = nc.dram_tensor("alpha_out", (S, BL, K), f32, kind="ExternalOutput").ap()

    AH = A // 128  # 2 partition tiles over the A axis

    with tile.TileContext(nc) as tc:
        with (
            tc.tile_pool(name="consts", bufs=1) as cpool,
            tc.tile_pool(name="sb", bufs=1) as spool,
            tc.tile_pool(name="ftk", bufs=2) as fpool,
            tc.tile_pool(name="gtq", bufs=2) as gpool,
            tc.tile_pool(name="psA", bufs=1, space="PSUM") as psA,
            tc.tile_pool(name="psK", bufs=2, space="PSUM") as psK,
            tc.tile_pool(name="ps1", bufs=1, space="PSUM") as ps1,
        ):
            # ---- constants / small tiles --------------------------------
            idt = cpool.tile([128, 128], bf16, tag="idt")
            masks.make_identity(nc, idt[:])
            phc = cpool.tile([128, 1], f32, tag="phc")
            nc.vector.memset(phc[:], HALF_PI)
            warm = cpool.tile([128, 1], f32, tag="warm")
            nc.scalar.activation(warm[:], phc[:], AF.Sin, scale=1.0)
            wps = ps1.tile([128, 128], f32, tag="attps", bufs=2, name="wps")
            for i in range(16):
                nc.tensor.matmul(wps[:], idt[:], idt[:],
                                 start=(i == 0), stop=(i == 15))


            Wbs = cpool.tile([128, AH], f32, tag="Wbs")
            nc.gpsimd.dma_start(Wbs[:], Wb_d.rearrange("(ah p) one -> p (ah one)", p=128))
            Ubs = cpool.tile([128, AH], f32, tag="Ubs")
            nc.gpsimd.dma_start(Ubs[:], Ub_d.rearrange("(ah p) one -> p (ah one)", p=128))
            vreps = cpool.tile([128, 2, AH, BL, S], bf16, tag="vreps")
            nc.gpsimd.dma_start(vreps[:], vrep_d[:])
            wlin = cpool.tile([128, QD // 128], bf16, tag="wlin")
            nc.gpsimd.dma_start(wlin[:], wlin_d[:])
            ulin = cpool.tile([128, KD // 128, S], bf16, tag="ulin")
            nc.gpsimd.dma_start(ulin[:], ulin_d[:])

            # ---- bulk inputs (b0 tk chain first: it gates the pipeline)
            UwTs = cpool.tile([128, KD // 128, A], bf16, tag="UwTs")
            nc.scalar.dma_start(UwTs[:], UwT_d.rearrange("(dh p) a -> p dh a", p=128))
            kTs = spool.tile([128, KD // 128, BL, K], bf16, tag="kTs")
            for dh in range(KD // 128):
                eng = nc.sync if (dh % 2 == 0) else nc.scalar
                eng.dma_start(kTs[:, dh, 0, :], kT_d[0, dh * 128:(dh + 1) * 128, :])
            WwTs = cpool.tile([128, QD // 128, A], bf16, tag="WwTs")
            nc.scalar.dma_start(WwTs[:], WwT_d.rearrange("(qh p) a -> p qh a", p=128))
            qTs = spool.tile([128, QD // 128, BL, S], bf16, tag="qTs")
            for b in range(BL):
                nc.scalar.dma_start(
                    qTs[:, :, b, :],
                    qT_d[b].rearrange("(qh p) s -> p qh s", p=128))
            for dh in range(KD // 128):
                eng = nc.sync if (dh % 2 == 0) else nc.scalar
                eng.dma_start(kTs[:, dh, 1, :], kT_d[1, dh * 128:(dh + 1) * 128, :])
            keysb = spool.tile([128, K // 128, BL, KD], bf16, tag="keysb")
            for b in range(BL):
                for kh in range(K // 128):
                    nc.gpsimd.dma_start(
                        keysb[:, kh, b, :],
                        keys_d[b, kh * 128:(kh + 1) * 128, :])
            m01 = spool.tile([S, BL, K], bf16, tag="m01")
            nc.gpsimd.dma_start(m01[:], m01_d[:])

            # ---- combined t tensor: t[:, ah, b, :K] = tk, [K:] = tq ------
            # (single tensor so each range-wrap / sine pass covers both
            # sides in one instruction)
            KS = K + S
            tcomb = spool.tile([128, AH, BL, KS], f32, tag="tcomb")
            tk = tcomb[:, :, :, :K]
            tq = tcomb[:, :, :, K:]
            for b in range(BL):
                for ah in range(AH):
                    ps = ps1.tile([128, S], f32, tag="tqps", bufs=2)
                    for qh in range(QD // 128):
                        nc.tensor.matmul(
                            ps[:],
                            WwTs[:, qh, ah * 128:(ah + 1) * 128],
                            qTs[:, qh, b, :],
                            start=(qh == 0), stop=(qh == QD // 128 - 1),
                        )
                    nc.vector.tensor_scalar_add(tq[:, ah, b, :], ps[:],
                                                Wbs[:, ah:ah + 1])

            # ---- P[s] = a0 * sum_a v_a tq[s,a]  (+vb), per b ------------
            Psb = []
            for b in range(BL):
                pps = ps1.tile([S, 1], f32, tag="tqps", bufs=2, name="pps")
                for qh in range(QD // 128):
                    nc.tensor.matmul(
                        pps[:],
                        qTs[:, qh, b, :],
                        wlin[:, qh:qh + 1],
                        start=(qh == 0), stop=(qh == QD // 128 - 1),
                    )
                pb = spool.tile([S, 1], f32, tag=f"Psb{b}", name=f"Psb{b}")
                nc.vector.tensor_scalar_add(pb[:], pps[:], float(pbias_val))
                Psb.append(pb)

            # ---- tk = Uw @ keys^T (+Ub), laid out (a, k) per b ----------
            for b in range(BL):
                for ah in range(AH):
                    psk = psK.tile([128, K], f32, tag="tkps")
                    for dh in range(KD // 128):
                        nc.tensor.matmul(
                            psk[:],
                            UwTs[:, dh, ah * 128:(ah + 1) * 128],
                            kTs[:, dh, b, :],
                            start=(dh == 0), stop=(dh == KD // 128 - 1),
                        )
                    nc.scalar.add(tk[:, ah, b, :], psk[:],
                                  Ubs[:, ah:ah + 1])

            # ---- alpha accumulation in PSUM, (s, k) per b ---------------
            alpha_ps = [psA.tile([S, K], f32, name=f"al{b}", tag=f"al{b}")
                        for b in range(BL)]

            # linear term:  alpha += a0 * sum_a v_a tk[k,a]  (via raw keys)
            for b in range(BL):
                for dh in range(KD // 128):
                    nc.tensor.matmul(
                        alpha_ps[b][:],
                        ulin[:, dh, :],
                        kTs[:, dh, b, :],
                        start=(dh == 0), stop=False,
                    )

            # sine terms.  ACT Sin is only accurate to |arg| ~3.6-3.7 rad, so
            # arguments are range-reduced with ADD_RANGE_WRAP chains applied
            # to the raw t-tensors (bound pi/w, period 2pi/w in t-units).
            # The cos chain wraps (t + bound/2), so sin(w*u_c) = cos(w*t).
            # For tiny coefficients the cos is taken directly from u_s with a
            # +pi/2 bias (|arg| <= pi + pi/2 where the sin spline error,
            # ~0.075 absolute, is still small enough after weighting by c_m).
            tmax = max(tmax_q, tmax_k)

            def wrap_chain(src_view, w_m, tag, with_shift, shape):
                # src_view/dst used via flattened free dims (custom-DVE ops
                # accept <=2 free dims)
                bnd = math.pi / w_m
                if with_shift:
                    nwrap = 1
                else:
                    if w_m * tmax <= 3.6:
                        return src_view, 0
                    nwrap = max(1, int(math.ceil((tmax / bnd - 1.0) / 2.0
                                                 - 1e-9)))
                cur = src_view
                for i in range(nwrap):
                    dst = fpool.tile(shape, f32,
                                     tag=f"{tag}{i}", name=f"{tag}{i}",
                                     bufs=(1 if len(shape) == 3 else None))
                    sh = (bnd / 2.0) if (with_shift and i == 0) else 0.0
                    if len(shape) == 4:
                        d_v = dst[:].rearrange("p a b k -> p (a b k)")
                        c_v = cur[:].rearrange("p a b k -> p (a b k)")
                    else:
                        d_v, c_v = dst[:], cur[:]  # (p, a, k): 2 free dims
                    nc.vector.add_range_wrap(d_v, c_v, sh, bnd, 2.0 * bnd)
                    cur = dst
                return cur, nwrap

            def emit_m(m, bs, stop_b=None):
                """Emit one harmonic's wraps/sines/scale/matmuls for batches
                `bs` (either all of them in one shot, or a single b)."""
                w_m = float(omegas[m])
                c_m = float(coefs[m])
                cos_direct = abs(c_m) <= 0.03
                split = len(bs) < BL
                sfx = f"b{bs[0]}" if split else ""
                if split:
                    src = tcomb[:, :, bs[0], :]
                    shape = [128, AH, KS]
                else:
                    src = tcomb
                    shape = [128, AH, BL, KS]
                u_s, _ = wrap_chain(src, w_m, "uks" + sfx, False, shape)
                fshape = [128, 2] + shape[1:]
                fsc = fpool.tile(fshape, bf16, tag="fsc" + sfx,
                                 name="fsc" + sfx,
                                 bufs=(1 if split else None))
                nc.scalar.activation(fsc[:, 0], u_s[:], AF.Sin, scale=w_m)
                if cos_direct:
                    nc.scalar.activation(fsc[:, 1], u_s[:], AF.Sin,
                                         scale=w_m, bias=phc[:])
                else:
                    u_c, _ = wrap_chain(u_s, w_m, "ukc" + sfx, True, shape)
                    nc.scalar.activation(fsc[:, 1], u_c[:], AF.Sin,
                                         scale=w_m)
                gshape = fshape[:-1] + [S]
                gsc = gpool.tile(gshape, bf16, tag="gsc" + sfx,
                                 name="gsc" + sfx,
                                 bufs=(1 if split else None))
                if split:
                    fq = fsc[:, :, :, K:]
                    vr = vreps[:, :, :, bs[0], :]
                else:
                    fq = fsc[:, :, :, :, K:]
                    vr = vreps[:]
                nc.vector.scalar_tensor_tensor(gsc[:], fq, c_m, vr,
                                               op0=OP.mult, op1=OP.mult)
                for b in bs:
                    for ah in range(AH):
                        if split:
                            g0, g1 = gsc[:, 0, ah, :], gsc[:, 1, ah, :]
                            f0, f1 = fsc[:, 0, ah, :K], fsc[:, 1, ah, :K]
                        else:
                            g0, g1 = gsc[:, 0, ah, b, :], gsc[:, 1, ah, b, :]
                            f0 = fsc[:, 0, ah, b, :K]
                            f1 = fsc[:, 1, ah, b, :K]
                        stop = (stop_b == b and ah == AH - 1)
                        nc.tensor.matmul(alpha_ps[b][:], g0, f1,
                                         start=False, stop=False)
                        nc.tensor.matmul(alpha_ps[b][:], g1, f0,
                                         start=False, stop=stop)

            # PE "heater" blocks: back-to-back tiny matmuls emitted between
            # harmonic groups.  They run whenever the next real matmul's
            # operands aren't ready yet, keeping the PE busy through the
            # ACT/DVE-wait gaps so the HAM clock gate stays at full rate
            # (throttled alpha matmuls cost 427-788ns vs 213ns warm).
            def pe_fill(n, j):
                fps = ps1.tile([128, 128], f32, tag="attps", bufs=2,
                               name=f"fill{j}")
                for i in range(n):
                    nc.tensor.matmul(fps[:], idt[:], idt[:],
                                     start=(i == 0), stop=(i == n - 1))

            # m=0 split per b (primes the pipeline as soon as each batch's
            # t-tensor is ready); middle m's batched; last m split per b so
            # b0's softmax overlaps b1's remaining work.
            emit_m(0, [0])
            emit_m(0, [1])
            pe_fill(14, 0)
            for m in range(1, M - 1):
                emit_m(m, list(range(BL)))
                pe_fill(14, m)
            emit_m(M - 1, [0], stop_b=0)
            emit_m(M - 1, [1], stop_b=1)
            # dummy exp on a const tile: triggers the exp table load while
            # the last alpha matmuls are still in flight
            nc.scalar.activation(warm[:], phc[:], AF.Exp)

            # ---- softmax + outputs per b --------------------------------
            for b in range(BL):
                nmax = spool.tile([S, 1], f32, tag=f"nmax{b}")
                nc.vector.tensor_reduce(nmax[:], alpha_ps[b][:], axis=AX.X,
                                        op=OP.max, negate=True)
                esb = spool.tile([S, K], bf16, tag=f"esb{b}")
                nc.scalar.activation(esb[:], alpha_ps[b][:], AF.Exp,
                                     bias=nmax[:], scale=1.0)
                expm = spool.tile([S, K], bf16, tag=f"expm{b}")
                sume = spool.tile([S, 1], f32, tag=f"sume{b}")
                nc.vector.scalar_tensor_tensor(expm[:], esb[:], 1.0,
                                               m01[:, b, :],
                                               op0=OP.mult, op1=OP.mult,
                                               accum_out=sume[:])
                rcp = spool.tile([S, 1], f32, tag=f"rcp{b}")
                nc.vector.reciprocal(rcp[:], sume[:])

                # alpha output = alpha + P[s] (+vb); -inf mask applied on
                # the host during output assembly
                alpsb = spool.tile([S, K], f32, tag=f"alp{b}")
                nc.scalar.activation(alpsb[:], alpha_ps[b][:], AF.Identity,
                                     bias=Psb[b][:])
                nc.scalar.dma_start(alp_d[:, b, :], alpsb[:])

                # transpose masked weights to (k, s)
                wT = spool.tile([128, K // 128, S], bf16, tag=f"wT{b}")
                for kh in range(K // 128):
                    tps = ps1.tile([128, S], bf16, tag="tqps", bufs=2, name="tps")
                    nc.tensor.transpose(tps[:],
                                        expm[:, kh * 128:(kh + 1) * 128],
                                        idt[:S, :S])
                    nc.vector.tensor_copy(wT[:, kh, :], tps[:])

                att_ps = ps1.tile([S, KD], f32, tag="attps", bufs=2)
                for kh in range(K // 128):
                    nc.tensor.matmul(
                        att_ps[:],
                        wT[:, kh, :],
                        keysb[:, kh, b, :],
                        start=(kh == 0), stop=(kh == K // 128 - 1),
                    )
                attsb = spool.tile([S, KD], f32, tag=f"att{b}")
                nc.scalar.activation(attsb[:], att_ps[:], AF.Copy,
                                     scale=rcp[:])
                nc.sync.dma_start(att_d[:, b, :], attsb[:])

    nc.compile()
    return nc


def _pick_table(zmax: float) -> int:
    for i, (z0, _, _, _) in enumerate(TANH_SIN_TABLES):
        if zmax <= z0:
            return i
    return len(TANH_SIN_TABLES) - 1


def kernel(queries, keys, null_mask, Ww, Wb, Uw, Ub, vw, vb):
    from concourse import bass_utils

    queries = np.asarray(queries, dtype=np.float32)
    keys = np.asarray(keys, dtype=np.float32)
    null_mask = np.asarray(null_mask)
    Ww = np.asarray(Ww, dtype=np.float32)
    Wb = np.asarray(Wb, dtype=np.float32)
    Uw = np.asarray(Uw, dtype=np.float32)
    Ub = np.asarray(Ub, dtype=np.float32)
    vw = np.asarray(vw, dtype=np.float32)
    vb = np.asarray(vb, dtype=np.float32)
    v = vw[0]  # (A,)
    vb_val = float(vb[0])

    # Host-side range check to pick the sine table (exact max|tq+tk| via
    # per-(b,a) extrema of tq over s and tk over k).
    tq_h = (queries.reshape(S * B, QD) @ Ww.T).reshape(S, B, A) + Wb
    tk_h = (keys.reshape(B * K, KD) @ Uw.T).reshape(B, K, A) + Ub
    zmax = max(
        float((tq_h.max(0) + tk_h.max(1)).max()),
        float(-(tq_h.min(0) + tk_h.min(1)).min()),
    )
    tidx = _pick_table(zmax)
    z0, omegas, coefs, a0 = TANH_SIN_TABLES[tidx]

    # alpha linear-term constants that are uniform over k fold into the
    # per-row bias: vb + a0*(v.Wb) + a0*(v.Ub)
    pbias = vb_val + float(a0 * (v @ Wb)) + float(a0 * (v @ Ub))

    tmax_q = float(np.abs(tq_h).max()) * 1.02
    tmax_k = float(np.abs(tk_h).max()) * 1.02
    key_ = (round(pbias, 12), tidx, round(tmax_q, 2), round(tmax_k, 2))
    if key_ not in _program_cache:
        _program_cache[key_] = _build_program(pbias, tidx, tmax_q, tmax_k)
    nc = _program_cache[key_]

    bf = ml_dtypes.bfloat16
    WwT = np.ascontiguousarray(Ww.T).astype(bf)          # (QD, A)
    UwT = np.ascontiguousarray(Uw.T).astype(bf)          # (KD, A)
    Wb_c = np.ascontiguousarray(Wb.reshape(A, 1))
    Ub_c = np.ascontiguousarray(Ub.reshape(A, 1))
    v_pa = np.ascontiguousarray(v.reshape(A // 128, 128).T)      # (128, AH)
    vrep = np.ascontiguousarray(
        np.broadcast_to(v_pa[None, :, :, None, None],
                        (2, 128, A // 128, BL, S)).transpose(1, 0, 2, 3, 4)
    ).astype(bf)
    wlin_v = (a0 * (v @ Ww)).reshape(QD // 128, 128).T   # (128, qh)
    wlin = np.ascontiguousarray(wlin_v).astype(bf)
    ulin_v = (a0 * (v @ Uw)).reshape(KD // 128, 128).T   # (128, dh)
    ulin = np.ascontiguousarray(
        np.broadcast_to(ulin_v[:, :, None], (128, KD // 128, S))
    ).astype(bf)

    in_maps = []
    for c in range(NCORES):
        bs = slice(c * BL, (c + 1) * BL)
        q_c = queries[:, bs, :]                       # (S, BL, QD)
        k_c = keys[bs]                                # (BL, K, KD)
        nm_c = null_mask[bs]                          # (BL, K)
        m01 = np.ascontiguousarray(
            np.broadcast_to((~nm_c).astype(bf)[None, :, :], (S, BL, K))
        )
        in_maps.append({
            "qT": np.ascontiguousarray(q_c.transpose(1, 2, 0)).astype(bf),
            "kT": np.ascontiguousarray(k_c.transpose(0, 2, 1)).astype(bf),
            "keysb": np.ascontiguousarray(k_c).astype(bf),
            "WwT": WwT,
            "UwT": UwT,
            "Wbc": Wb_c,
            "Ubc": Ub_c,
            "vrep": vrep,
            "wlin": wlin,
            "ulin": ulin,
            "mask01": m01,
        })

    res = bass_utils.run_bass_kernel_spmd(
        nc, in_maps, core_ids=list(range(NCORES)))
    global last_result
    last_result = res

    att = np.empty((S, B, KD), np.float32)
    alpha = np.empty((S, B, K), np.float32)
    for c in range(NCORES):
        bs = slice(c * BL, (c + 1) * BL)
        att[:, bs, :] = res.results[c]["att_out"]
        alpha[:, bs, :] = res.results[c]["alpha_out"]
    alpha[np.broadcast_to(null_mask[None, :, :], alpha.shape)] = -np.inf
    return att, alpha
